# revision 15
# baseline (speedup 1.0000x reference)
"""Trainium2 Bass kernel for nn_EnsembleDynamicModel.

Ensemble MLP: E=7 members, x=[state(32)|action(8)] -> 256 -> 256 -> 256 -> 128
-> {mu(32), log_sigma(32)} with swish hidden activations, soft-clamped
log_sigma -> sigma=exp(.), and mu += state residual.

Strategy: data-parallel over the batch axis. Each of the 8 NeuronCores gets
B/8 = 4096 batch rows; ensemble weights are replicated. On-chip layout keeps
activations feature-major ([feature, batch]) so the contraction dim of every
GEMM sits on SBUF partitions:

    h_out[o, n] = sum_k W[k, o] * h_in[k, n]   (lhsT = W tile, rhs = h_in)

The host pre-transposes x once (cheap, 5 MB) and post-transposes the two
[E*32, B/8] outputs back.

Engines:
  PE   : whole GEMM chain, fp32 (1 col/cycle regardless of dtype on trn2).
  ACT  : swish fused with the bias add (Silu(psum*1 + b)), and the
         softplus/softplus/exp soft-clamp chain for sigma, run on tiles that
         pack 4 ensembles' sigma rows to use all 128 lanes.
  DVE  : mu head = one fused affine_then_add (psum + bmu + state), plus the
         quadrant-shifting copies that pack sigma rows.
"""

import os
import sys
import numpy as np
from contextlib import ExitStack

# concourse ships with the container image (also on PYTHONPATH via axon_site).
for _p in ("/opt/trn_rl_repo", "/root/.axon_site/_ro/trn_rl_repo"):
    if os.path.isdir(_p) and _p not in sys.path:
        sys.path.append(_p)

import concourse.bass as bass  # noqa: E402
import concourse.tile as tile  # noqa: E402
import concourse.mybir as mybir  # noqa: E402
from concourse import bacc  # noqa: E402
from concourse.bass_utils import run_bass_kernel_spmd  # noqa: E402
from concourse import bass_utils as _bu  # noqa: E402

# Consecutive matmuls here reuse the same stationary weights 4x; walrus's
# LDWEIGHTS dedup (off by default) removes the redundant reloads.
if not getattr(_bu, "_ldw_opt_patched", False):
    _orig_run_command = _bu.run_command

    def _run_command_ldw(argv, **kw):
        argv = ["--enable-ldw-opt=true" if a == "--enable-ldw-opt=false" else a
                for a in argv]
        return _orig_run_command(argv, **kw)

    _bu.run_command = _run_command_ldw
    _bu._ldw_opt_patched = True

F32 = mybir.dt.float32
# PE matmul dtype: float32r is the single-pass reduced-precision fp32 mode
# (4x faster than fp32's two half-rate passes at free dim >= 256); same
# 4-byte layout, so it's a pure AP bitcast on the matmul operands.
MMDT = mybir.dt.float32r
AF = mybir.ActivationFunctionType

E = 7
B = 32768
S = 32
A = 8
DIN = S + A            # 40
HID = [256, 256, 256, 128]
NCORES = 8
BL = B // NCORES       # 4096 batch rows per core
CH = 2048              # batch chunk per psum tile (4 PSUM banks fp32)
NSUB = 512             # one matmul's free dim (1 PSUM bank fp32)
NCHUNK = BL // CH      # 2
NJ = CH // NSUB        # 4
NCONST = 8             # const columns per ensemble member


def _build_kernel(ctx, tc, io, act=AF.Silu):
    nc = tc.nc
    cpool = ctx.enter_context(tc.tile_pool(name="cpool", bufs=1))
    hpool = ctx.enter_context(tc.tile_pool(name="hpool", bufs=1))
    wpool = ctx.enter_context(tc.tile_pool(name="wpool", bufs=2))
    pspool = ctx.enter_context(tc.tile_pool(name="pspool", bufs=2, space="PSUM"))
    mupool = ctx.enter_context(tc.tile_pool(name="mupool", bufs=3))
    sgpool = ctx.enter_context(tc.tile_pool(name="sgpool", bufs=3))

    def load_weights(e):
        w0 = wpool.tile([DIN, 256], F32, tag="w0", name="w0")
        nc.sync.dma_start(w0[:].bitcast(MMDT), io["w0"][e].bitcast(MMDT))
        w1, w2, w3 = [], [], []
        for k in range(2):
            t = wpool.tile([128, 256], F32, tag=f"w1_{k}", name=f"w1_{k}")
            nc.sync.dma_start(t[:].bitcast(MMDT),
                              io["w1"][e, k * 128:(k + 1) * 128, :].bitcast(MMDT))
            w1.append(t)
            t = wpool.tile([128, 256], F32, tag=f"w2_{k}", name=f"w2_{k}")
            nc.sync.dma_start(t[:].bitcast(MMDT),
                              io["w2"][e, k * 128:(k + 1) * 128, :].bitcast(MMDT))
            w2.append(t)
            t = wpool.tile([128, 128], F32, tag=f"w3_{k}", name=f"w3_{k}")
            nc.sync.dma_start(t[:].bitcast(MMDT),
                              io["w3"][e, k * 128:(k + 1) * 128, :].bitcast(MMDT))
            w3.append(t)
        wh = wpool.tile([128, 64], F32, tag="wh", name="wh")
        nc.sync.dma_start(wh[:].bitcast(MMDT), io["wh"][e].bitcast(MMDT))
        return w0, w1, w2, w3, wh

    # --- startup order: w0(e0), first xt chunk, consts, rest ---
    xt = cpool.tile([DIN, BL], F32, tag="xt")
    w0e = wpool.tile([DIN, 256], F32, tag="w0", name="w0")
    nc.sync.dma_start(w0e[:].bitcast(MMDT), io["w0"][0].bitcast(MMDT))
    for j in range(NJ):
        js = slice(j * NSUB, (j + 1) * NSUB)
        nc.sync.dma_start(xt[:, js].bitcast(MMDT), io["xt"][:, js].bitcast(MMDT))
    cns = cpool.tile([128, E * NCONST], F32, tag="cns")
    nc.sync.dma_start(cns[:], io["cns"])
    sgc = cpool.tile([128, 2], F32, tag="sgc")
    nc.sync.dma_start(sgc[:], io["sgc"])
    for j in range(NJ):
        js = slice(CH + j * NSUB, CH + (j + 1) * NSUB)
        nc.sync.dma_start(xt[:, js].bitcast(MMDT), io["xt"][:, js].bitcast(MMDT))
    resid = cpool.tile([64, BL], F32, tag="resid")
    nc.sync.dma_start(resid[:], io["resid"])

    # --- activation buffers, ping-pong between layers ---
    hA = [hpool.tile([128, BL], F32, tag=f"hA{i}", name=f"hA{i}") for i in range(2)]
    hB = [hpool.tile([128, BL], F32, tag=f"hB{i}", name=f"hB{i}") for i in range(2)]

    def gemm_layer(h_in, w_tiles, m_tiles, h_out, bias_cols, e):
        """h_out[mt][:, :] = Silu(sum_kt w[kt].T @ h_in[kt] + b)."""
        nkt = len(h_in)
        for c in range(NCHUNK):
            for mt in range(m_tiles):
                ps = pspool.tile([128, CH], F32, tag="ps", name="ps")
                for kt in range(nkt):
                    for j in range(NJ):
                        ncol = slice(c * CH + j * NSUB, c * CH + (j + 1) * NSUB)
                        nc.tensor.matmul(
                            ps[:, j * NSUB:(j + 1) * NSUB],
                            w_tiles[kt][:, mt * 128:(mt + 1) * 128].bitcast(MMDT),
                            h_in[kt][:, ncol].bitcast(MMDT),
                            start=(kt == 0),
                            stop=(kt == nkt - 1),
                            skip_group_check=True,
                        )
                bcol = e * NCONST + bias_cols[mt]
                nc.scalar.activation(
                    h_out[mt][:, c * CH:(c + 1) * CH].bitcast(MMDT), ps[:, :],
                    act, bias=cns[:, bcol:bcol + 1],
                )

    # Software pipeline over ensembles: L0(e+1) is emitted between L3(e)
    # and head(e) so the PE always has independent matmul work while the
    # head psums drain on the DVE.
    w_cur = None
    for e in range(E):
        if e == 0:
            w1, w2, w3 = [], [], []
            for k in range(2):
                t = wpool.tile([128, 256], F32, tag=f"w1_{k}", name=f"w1_{k}")
                nc.sync.dma_start(
                    t[:].bitcast(MMDT),
                    io["w1"][e, k * 128:(k + 1) * 128, :].bitcast(MMDT))
                w1.append(t)
                t = wpool.tile([128, 256], F32, tag=f"w2_{k}", name=f"w2_{k}")
                nc.sync.dma_start(
                    t[:].bitcast(MMDT),
                    io["w2"][e, k * 128:(k + 1) * 128, :].bitcast(MMDT))
                w2.append(t)
                t = wpool.tile([128, 128], F32, tag=f"w3_{k}", name=f"w3_{k}")
                nc.sync.dma_start(
                    t[:].bitcast(MMDT),
                    io["w3"][e, k * 128:(k + 1) * 128, :].bitcast(MMDT))
                w3.append(t)
            wh = wpool.tile([128, 64], F32, tag="wh", name="wh")
            nc.sync.dma_start(wh[:].bitcast(MMDT), io["wh"][e].bitcast(MMDT))
            w_cur = (w0e, w1, w2, w3, wh)
            gemm_layer([xt], [w0e], 2, hA, (0, 1), 0)   # L0 of e=0
        w0, w1, w2, w3, wh = w_cur

        # --- the GEMM chain, feature-major ---
        gemm_layer(hA, w1, 2, hB, (2, 3), e)           # 256  -> 256
        gemm_layer(hB, w2, 2, hA, (4, 5), e)           # 256  -> 256
        gemm_layer(hA, w3, 1, hB, (6,), e)             # 256  -> 128 (hB[0])
        h3 = hB[0]

        if e + 1 < E:
            w_cur = load_weights(e + 1)
            gemm_layer([xt], [w_cur[0]], 2, hA, (0, 1), e + 1)  # L0 of e+1

        # --- heads: one [128,64] matmul -> rows 0-31 mu_pre, 32-63 sig_pre ---
        # soft_clamp(y) = max-sp(max-y) then min+sp(.-min); exp of it
        # collapses algebraically:  sigma = exp(min) + exp(max)*sigmoid(y-max)
        for c in range(NCHUNK):
            cs = slice(c * CH, (c + 1) * CH)
            ps = pspool.tile([64, CH], F32, tag="ps", name="psh")
            for j in range(NJ):
                ncol = slice(c * CH + j * NSUB, c * CH + (j + 1) * NSUB)
                nc.tensor.matmul(
                    ps[:, j * NSUB:(j + 1) * NSUB],
                    wh[:, :].bitcast(MMDT), h3[:, ncol].bitcast(MMDT),
                    start=True, stop=True,
                )
            # single fused DVE op drains the whole head psum:
            #   rows 0-31:  mu  = psum + bmu + state
            #   rows 32-63: y'  = psum + (bsig - max) + 0
            bcol = e * NCONST + 7
            hd = sgpool.tile([64, CH], F32, tag="hd", name="hd")
            nc.vector.affine_then_add(
                hd[:, :], ps[:, :], resid[:, cs], 1.0,
                cns[0:64, bcol:bcol + 1],
            )
            nc.sync.dma_start(io["mu"][e * 32:(e + 1) * 32, cs], hd[0:32, :])
            # sigmoid via tanh (same ACT table set as Silu -> no table swap):
            # sigmoid(y') = 0.5*tanh(y'/2) + 0.5
            sg = sgpool.tile([64, CH], F32, tag="sg", name="sg")
            nc.scalar.activation(sg[32:64, :], hd[32:64, :], AF.Tanh, scale=0.5)
            # sigma = tanh_out*(exp(max)/2) + (exp(min) + exp(max)/2)
            sig = sgpool.tile([64, CH], F32, tag="sig", name="sig")
            nc.vector.tensor_scalar(
                sig[32:64, :], sg[32:64, :],
                sgc[32:64, 0:1], sgc[32:64, 1:2],
                mybir.AluOpType.mult, mybir.AluOpType.add,
            )
            nc.sync.dma_start(io["sig"][e * 32:(e + 1) * 32, cs], sig[32:64, :])


def build_program(act=AF.Silu):
    nc = bacc.Bacc(
        "TRN2", target_bir_lowering=False, debug=False, num_devices=NCORES
    )
    io = {
        "xt": nc.dram_tensor("xt", [DIN, BL], F32, kind="ExternalInput").ap(),
        "resid": nc.dram_tensor(
            "resid", [64, BL], F32, kind="ExternalInput"
        ).ap(),
        "w0": nc.dram_tensor("w0", [E, DIN, 256], F32, kind="ExternalInput").ap(),
        "w1": nc.dram_tensor("w1", [E, 256, 256], F32, kind="ExternalInput").ap(),
        "w2": nc.dram_tensor("w2", [E, 256, 256], F32, kind="ExternalInput").ap(),
        "w3": nc.dram_tensor("w3", [E, 256, 128], F32, kind="ExternalInput").ap(),
        "wh": nc.dram_tensor("wh", [E, 128, 64], F32, kind="ExternalInput").ap(),
        "cns": nc.dram_tensor(
            "cns", [128, E * NCONST], F32, kind="ExternalInput"
        ).ap(),
        "sgc": nc.dram_tensor("sgc", [128, 2], F32, kind="ExternalInput").ap(),
        "mu": nc.dram_tensor("mu", [E * 32, BL], F32, kind="ExternalOutput").ap(),
        "sig": nc.dram_tensor("sig", [E * 32, BL], F32, kind="ExternalOutput").ap(),
    }
    with tile.TileContext(nc) as tc, ExitStack() as ctx:
        _build_kernel(ctx, tc, io, act=act)
    nc.compile()
    return nc


def host_prep(state, action, W0, b0, W1, b1, W2, b2, W3, b3,
              Wmu, bmu, Wsig, bsig, max_logstd, min_logstd):
    """Full inputs -> (shared input map, per-core xt shards)."""
    f = lambda a: np.ascontiguousarray(np.asarray(a), dtype=np.float32)
    state, action = f(state), f(action)
    xt_full = np.ascontiguousarray(
        np.concatenate([state, action], axis=1).T
    )  # [40, B]
    W0, W1, W2, W3 = f(W0), f(W1), f(W2), f(W3)
    wh = np.ascontiguousarray(np.concatenate([f(Wmu), f(Wsig)], axis=2))
    b0, b1, b2, b3 = f(b0), f(b1), f(b2), f(b3)
    bmu, bsig = f(bmu), f(bsig)
    mx, mn = f(max_logstd), f(min_logstd)

    cns = np.zeros((128, E * NCONST), np.float32)
    for e in range(E):
        c = e * NCONST
        cns[:, c + 0] = b0[e, :128]
        cns[:, c + 1] = b0[e, 128:]
        cns[:, c + 2] = b1[e, :128]
        cns[:, c + 3] = b1[e, 128:]
        cns[:, c + 4] = b2[e, :128]
        cns[:, c + 5] = b2[e, 128:]
        cns[:, c + 6] = b3[e, :]
        cns[0:32, c + 7] = bmu[e]
        cns[32:64, c + 7] = bsig[e] - mx   # sigma-head drain bias

    sgc = np.zeros((128, 2), np.float32)
    sgc[32:64, 0] = np.exp(mx) / 2
    sgc[32:64, 1] = np.exp(mn) + np.exp(mx) / 2

    shared = {
        "w0": W0, "w1": W1, "w2": W2, "w3": W3, "wh": wh,
        "cns": cns, "sgc": sgc,
    }
    resid_full = np.zeros((64, B), np.float32)
    resid_full[0:32] = xt_full[0:32]
    shards = [
        {
            "xt": np.ascontiguousarray(xt_full[:, c * BL:(c + 1) * BL]),
            "resid": np.ascontiguousarray(resid_full[:, c * BL:(c + 1) * BL]),
        }
        for c in range(NCORES)
    ]
    return shared, shards


def host_post(results):
    """Per-core {mu,sig} [E*32, BL] -> (mu [E,B,32], sigma [E,B,32])."""
    mu = np.empty((E, B, 32), np.float32)
    sigma = np.empty((E, B, 32), np.float32)
    for c in range(NCORES):
        bs = slice(c * BL, (c + 1) * BL)
        mu[:, bs, :] = results[c]["mu"].reshape(E, 32, BL).transpose(0, 2, 1)
        sigma[:, bs, :] = results[c]["sig"].reshape(E, 32, BL).transpose(0, 2, 1)
    return mu, sigma


_PROGRAM = None


def _get_program():
    global _PROGRAM
    if _PROGRAM is None:
        _PROGRAM = build_program()
    return _PROGRAM


def kernel(**inputs):
    nc = _get_program()
    shared, shards = host_prep(**inputs)
    in_maps = [{**shared, **shards[c]} for c in range(NCORES)]
    res = run_bass_kernel_spmd(nc, in_maps, list(range(NCORES)))
    return host_post(res.results)


# revision 18
# speedup vs baseline: 1.0479x; 1.0479x over previous
"""Trainium2 Bass kernel for nn_EnsembleDynamicModel.

Ensemble MLP: E=7 members, x=[state(32)|action(8)] -> 256 -> 256 -> 256 -> 128
-> {mu(32), log_sigma(32)} with swish hidden activations, soft-clamped
log_sigma -> sigma=exp(.), and mu += state residual.

Strategy: data-parallel over the batch axis. Each of the 8 NeuronCores gets
B/8 = 4096 batch rows; ensemble weights are replicated. On-chip layout keeps
activations feature-major ([feature, batch]) so the contraction dim of every
GEMM sits on SBUF partitions:

    h_out[o, n] = sum_k W[k, o] * h_in[k, n]   (lhsT = W tile, rhs = h_in)

The host pre-transposes x once (cheap, 5 MB) and post-transposes the two
[E*32, B/8] outputs back.

Engines:
  PE   : whole GEMM chain. Storage dtype for weights/activations is bf16 by
         default (full-rate 1 column/cycle at the warm 2.4 GHz clock + fast
         weight load); fp32 storage with float32r matmul views is the
         higher-precision fallback (~427ns per 512-col matmul, SBUF-BW
         limited). PSUM accumulation is fp32 either way.
  ACT  : swish fused with the bias add (Silu(psum + b)); the sigma head's
         sigmoid runs as Tanh (same ACT table set as Silu, so the function
         table never swaps): sigmoid(z) = 0.5*tanh(z/2) + 0.5.
  DVE  : one fused affine_then_add drains each head psum (mu = psum + bmu +
         state on rows 0-31, sigma-preact + (bsig-max) on rows 32-63), plus
         the final sigma scale/offset.

The reference's soft_clamp+exp collapses exactly:
    sigma = exp(min) + exp(max) * sigmoid(y - max).

Ensembles are software-pipelined: L0(e+1) is emitted between L3(e) and
head(e) so the PE always has independent matmul work while head psums drain.
"""

import os
import sys
import numpy as np
from contextlib import ExitStack

# concourse ships with the container image (also on PYTHONPATH via axon_site).
for _p in ("/opt/trn_rl_repo", "/root/.axon_site/_ro/trn_rl_repo"):
    if os.path.isdir(_p) and _p not in sys.path:
        sys.path.append(_p)

import ml_dtypes  # noqa: E402
import concourse.bass as bass  # noqa: E402
import concourse.tile as tile  # noqa: E402
import concourse.mybir as mybir  # noqa: E402
from concourse import bacc  # noqa: E402
from concourse.bass_utils import run_bass_kernel_spmd  # noqa: E402
from concourse import bass_utils as _bu  # noqa: E402

USE_BF16 = True

# Consecutive matmuls here reuse the same stationary weights 4x; walrus's
# LDWEIGHTS dedup (off by default) removes the redundant reloads. Only safe
# for 4-byte weight loads — bf16's fast-weight-load path rejects the opt.
if not USE_BF16 and not getattr(_bu, "_ldw_opt_patched", False):
    _orig_run_command = _bu.run_command

    def _run_command_ldw(argv, **kw):
        argv = ["--enable-ldw-opt=true" if a == "--enable-ldw-opt=false" else a
                for a in argv]
        return _orig_run_command(argv, **kw)

    _bu.run_command = _run_command_ldw
    _bu._ldw_opt_patched = True

F32 = mybir.dt.float32
AF = mybir.ActivationFunctionType

if USE_BF16:
    STORE = mybir.dt.bfloat16      # weights + hidden activations storage
    NP_STORE = ml_dtypes.bfloat16
    _mmv = lambda ap: ap           # matmul reads the tiles natively
else:
    STORE = F32                    # fp32 storage, float32r matmul views
    NP_STORE = np.float32
    _mmv = lambda ap: ap.bitcast(mybir.dt.float32r)

E = 7
B = 32768
S = 32
A = 8
DIN = S + A            # 40
NCORES = 8
BL = B // NCORES       # 4096 batch rows per core
CH = 2048              # batch chunk per psum tile (4 PSUM banks fp32)
NSUB = 512             # one matmul's free dim (1 PSUM bank fp32)
NCHUNK = BL // CH      # 2
NJ = CH // NSUB        # 4
NCONST = 8             # const columns per ensemble member


def _build_kernel(ctx, tc, io, act=AF.Silu):
    nc = tc.nc
    cpool = ctx.enter_context(tc.tile_pool(name="cpool", bufs=1))
    hpool = ctx.enter_context(tc.tile_pool(name="hpool", bufs=1))
    wpool = ctx.enter_context(tc.tile_pool(name="wpool", bufs=2))
    pspool = ctx.enter_context(tc.tile_pool(name="pspool", bufs=2, space="PSUM"))
    sgpool = ctx.enter_context(tc.tile_pool(name="sgpool", bufs=3))

    def load_weights(e, first=False):
        w0 = wpool.tile([DIN, 256], STORE, tag="w0", name="w0")
        nc.sync.dma_start(_mmv(w0[:]), _mmv(io["w0"][e]))
        if first:
            # startup: xt chunk 0 right after w0 so L0 can begin ASAP
            for j in range(NJ):
                js = slice(j * NSUB, (j + 1) * NSUB)
                nc.sync.dma_start(_mmv(xt[:, js]), _mmv(io["xt"][:, js]))
            nc.sync.dma_start(cns[:], io["cns"])
            nc.sync.dma_start(sgc[:], io["sgc"])
        w1, w2, w3 = [], [], []
        for k in range(2):
            t = wpool.tile([128, 256], STORE, tag=f"w1_{k}", name=f"w1_{k}")
            nc.sync.dma_start(_mmv(t[:]),
                              _mmv(io["w1"][e, k * 128:(k + 1) * 128, :]))
            w1.append(t)
            t = wpool.tile([128, 256], STORE, tag=f"w2_{k}", name=f"w2_{k}")
            nc.sync.dma_start(_mmv(t[:]),
                              _mmv(io["w2"][e, k * 128:(k + 1) * 128, :]))
            w2.append(t)
            t = wpool.tile([128, 128], STORE, tag=f"w3_{k}", name=f"w3_{k}")
            nc.sync.dma_start(_mmv(t[:]),
                              _mmv(io["w3"][e, k * 128:(k + 1) * 128, :]))
            w3.append(t)
        wh = wpool.tile([128, 64], STORE, tag="wh", name="wh")
        nc.sync.dma_start(_mmv(wh[:]), _mmv(io["wh"][e]))
        if first:
            for j in range(NJ):
                js = slice(CH + j * NSUB, CH + (j + 1) * NSUB)
                nc.sync.dma_start(_mmv(xt[:, js]), _mmv(io["xt"][:, js]))
            nc.sync.dma_start(resid[:], io["resid"])
        return w0, w1, w2, w3, wh

    xt = cpool.tile([DIN, BL], STORE, tag="xt")
    cns = cpool.tile([128, E * NCONST], F32, tag="cns")
    sgc = cpool.tile([128, 2], F32, tag="sgc")
    resid = cpool.tile([64, BL], F32, tag="resid")

    # --- activation buffers, ping-pong between layers ---
    hA = [hpool.tile([128, BL], STORE, tag=f"hA{i}", name=f"hA{i}")
          for i in range(2)]
    hB = [hpool.tile([128, BL], STORE, tag=f"hB{i}", name=f"hB{i}")
          for i in range(2)]

    def gemm_layer(h_in, w_tiles, m_tiles, h_out, bias_cols, e):
        """h_out[mt][:, :] = act(sum_kt w[kt].T @ h_in[kt] + b)."""
        nkt = len(h_in)
        for c in range(NCHUNK):
            for mt in range(m_tiles):
                ps = pspool.tile([128, CH], F32, tag="ps", name="ps")
                for kt in range(nkt):
                    for j in range(NJ):
                        ncol = slice(c * CH + j * NSUB, c * CH + (j + 1) * NSUB)
                        nc.tensor.matmul(
                            ps[:, j * NSUB:(j + 1) * NSUB],
                            _mmv(w_tiles[kt][:, mt * 128:(mt + 1) * 128]),
                            _mmv(h_in[kt][:, ncol]),
                            start=(kt == 0),
                            stop=(kt == nkt - 1),
                            skip_group_check=True,
                        )
                bcol = e * NCONST + bias_cols[mt]
                nc.scalar.activation(
                    _mmv(h_out[mt][:, c * CH:(c + 1) * CH]), ps[:, :],
                    act, bias=cns[:, bcol:bcol + 1],
                )

    # Software pipeline over ensembles: L0(e+1) is emitted between L3(e)
    # and head(e) so the PE always has independent matmul work while the
    # head psums drain on the DVE.
    w_cur = None
    for e in range(E):
        if e == 0:
            w_cur = load_weights(0, first=True)
            gemm_layer([xt], [w_cur[0]], 2, hA, (0, 1), 0)   # L0 of e=0
        w0, w1, w2, w3, wh = w_cur

        # --- the GEMM chain, feature-major ---
        gemm_layer(hA, w1, 2, hB, (2, 3), e)           # 256  -> 256
        gemm_layer(hB, w2, 2, hA, (4, 5), e)           # 256  -> 256
        gemm_layer(hA, w3, 1, hB, (6,), e)             # 256  -> 128 (hB[0])
        h3 = hB[0]

        if e + 1 < E:
            w_cur = load_weights(e + 1)
            gemm_layer([xt], [w_cur[0]], 2, hA, (0, 1), e + 1)  # L0 of e+1

        # --- heads: one [128,64] matmul -> rows 0-31 mu_pre, 32-63 sig_pre ---
        # The reference's soft_clamp+exp collapses exactly to
        #   sigma = exp(min) + exp(max)*sigmoid(y - max)
        for c in range(NCHUNK):
            cs = slice(c * CH, (c + 1) * CH)
            ps = pspool.tile([64, CH], F32, tag="ps", name="psh")
            for j in range(NJ):
                ncol = slice(c * CH + j * NSUB, c * CH + (j + 1) * NSUB)
                nc.tensor.matmul(
                    ps[:, j * NSUB:(j + 1) * NSUB],
                    _mmv(wh[:, :]), _mmv(h3[:, ncol]),
                    start=True, stop=True,
                )
            # single fused DVE op drains the whole head psum:
            #   rows 0-31:  mu = psum + bmu + state
            #   rows 32-63: y' = psum + (bsig - max) + 0
            bcol = e * NCONST + 7
            hd = sgpool.tile([64, CH], F32, tag="hd", name="hd")
            nc.vector.affine_then_add(
                hd[:, :], ps[:, :], resid[:, cs], 1.0,
                cns[0:64, bcol:bcol + 1],
            )
            nc.sync.dma_start(io["mu"][e * 32:(e + 1) * 32, cs], hd[0:32, :])
            # sigmoid via tanh (same ACT table set as Silu -> no table swap):
            # sigmoid(y') = 0.5*tanh(y'/2) + 0.5
            sg = sgpool.tile([64, CH], F32, tag="sg", name="sg")
            nc.scalar.activation(sg[32:64, :], hd[32:64, :], AF.Tanh, scale=0.5)
            # sigma = tanh_out*(exp(max)/2) + (exp(min) + exp(max)/2)
            sig = sgpool.tile([64, CH], F32, tag="sig", name="sig")
            nc.vector.tensor_scalar(
                sig[32:64, :], sg[32:64, :],
                sgc[32:64, 0:1], sgc[32:64, 1:2],
                mybir.AluOpType.mult, mybir.AluOpType.add,
            )
            nc.sync.dma_start(io["sig"][e * 32:(e + 1) * 32, cs], sig[32:64, :])


def build_program(act=AF.Silu):
    nc = bacc.Bacc(
        "TRN2", target_bir_lowering=False, debug=False, num_devices=NCORES
    )
    io = {
        "xt": nc.dram_tensor("xt", [DIN, BL], STORE,
                             kind="ExternalInput").ap(),
        "resid": nc.dram_tensor("resid", [64, BL], F32,
                                kind="ExternalInput").ap(),
        "w0": nc.dram_tensor("w0", [E, DIN, 256], STORE,
                             kind="ExternalInput").ap(),
        "w1": nc.dram_tensor("w1", [E, 256, 256], STORE,
                             kind="ExternalInput").ap(),
        "w2": nc.dram_tensor("w2", [E, 256, 256], STORE,
                             kind="ExternalInput").ap(),
        "w3": nc.dram_tensor("w3", [E, 256, 128], STORE,
                             kind="ExternalInput").ap(),
        "wh": nc.dram_tensor("wh", [E, 128, 64], STORE,
                             kind="ExternalInput").ap(),
        "cns": nc.dram_tensor("cns", [128, E * NCONST], F32,
                              kind="ExternalInput").ap(),
        "sgc": nc.dram_tensor("sgc", [128, 2], F32, kind="ExternalInput").ap(),
        "mu": nc.dram_tensor("mu", [E * 32, BL], F32,
                             kind="ExternalOutput").ap(),
        "sig": nc.dram_tensor("sig", [E * 32, BL], F32,
                              kind="ExternalOutput").ap(),
    }
    with tile.TileContext(nc) as tc, ExitStack() as ctx:
        _build_kernel(ctx, tc, io, act=act)
    nc.compile()
    return nc


def host_prep(state, action, W0, b0, W1, b1, W2, b2, W3, b3,
              Wmu, bmu, Wsig, bsig, max_logstd, min_logstd):
    """Full inputs -> (shared input map, per-core shard maps)."""
    f = lambda a: np.ascontiguousarray(np.asarray(a), dtype=np.float32)
    g = lambda a: np.ascontiguousarray(np.asarray(a, dtype=np.float32)
                                       .astype(NP_STORE))
    state, action = f(state), f(action)
    xt_full = np.ascontiguousarray(
        np.concatenate([state, action], axis=1).T
    )  # [40, B] fp32
    wh = np.concatenate([f(Wmu), f(Wsig)], axis=2)
    b0, b1, b2, b3 = f(b0), f(b1), f(b2), f(b3)
    bmu, bsig = f(bmu), f(bsig)
    mx, mn = f(max_logstd), f(min_logstd)

    cns = np.zeros((128, E * NCONST), np.float32)
    for e in range(E):
        c = e * NCONST
        cns[:, c + 0] = b0[e, :128]
        cns[:, c + 1] = b0[e, 128:]
        cns[:, c + 2] = b1[e, :128]
        cns[:, c + 3] = b1[e, 128:]
        cns[:, c + 4] = b2[e, :128]
        cns[:, c + 5] = b2[e, 128:]
        cns[:, c + 6] = b3[e, :]
        cns[0:32, c + 7] = bmu[e]
        cns[32:64, c + 7] = bsig[e] - mx   # sigma-head drain bias

    sgc = np.zeros((128, 2), np.float32)
    sgc[32:64, 0] = np.exp(mx) / 2
    sgc[32:64, 1] = np.exp(mn) + np.exp(mx) / 2

    shared = {
        "w0": g(W0), "w1": g(W1), "w2": g(W2), "w3": g(W3), "wh": g(wh),
        "cns": cns, "sgc": sgc,
    }
    resid_full = np.zeros((64, B), np.float32)
    resid_full[0:32] = xt_full[0:32]
    xt_store = xt_full.astype(NP_STORE)
    shards = [
        {
            "xt": np.ascontiguousarray(xt_store[:, c * BL:(c + 1) * BL]),
            "resid": np.ascontiguousarray(resid_full[:, c * BL:(c + 1) * BL]),
        }
        for c in range(NCORES)
    ]
    return shared, shards


def host_post(results):
    """Per-core {mu,sig} [E*32, BL] -> (mu [E,B,32], sigma [E,B,32])."""
    mu = np.empty((E, B, 32), np.float32)
    sigma = np.empty((E, B, 32), np.float32)
    for c in range(NCORES):
        bs = slice(c * BL, (c + 1) * BL)
        mu[:, bs, :] = results[c]["mu"].reshape(E, 32, BL).transpose(0, 2, 1)
        sigma[:, bs, :] = results[c]["sig"].reshape(E, 32, BL).transpose(0, 2, 1)
    return mu, sigma


_PROGRAM = None


def _get_program():
    global _PROGRAM
    if _PROGRAM is None:
        _PROGRAM = build_program()
    return _PROGRAM


def kernel(**inputs):
    nc = _get_program()
    shared, shards = host_prep(**inputs)
    in_maps = [{**shared, **shards[c]} for c in range(NCORES)]
    res = run_bass_kernel_spmd(nc, in_maps, list(range(NCORES)))
    return host_post(res.results)
